# revision 1
# baseline (speedup 1.0000x reference)
"""Trainium2 Bass kernel for nn_Block_7584912245089 (GNN message passing block).

Self-contained: host-side index/weight prep + an SPMD Bass program run on 8
NeuronCores via run_bass_kernel_spmd.

Algorithm (per core k; s = k//4 selects the sample, j = k%4 the row band):
- Nodes are resharded into 8 shards of 2050 rows (2 pole slots + 16 grid rows
  of 128 cols), stored in a padded "table" of 16400 rows x 576 f32
  (xp[512] | a_src[4] | a_dst[4] | pad). Gather indices are int16 row ids.
- GAT layer = per-dst-tile (128 nodes) streaming: one dma_gather per tile
  brings xp rows of all in-edges (sorted by dst) plus a final "self" chunk
  with the tile's own rows; attention softmax is computed WITHOUT the
  max-subtraction (mathematically identical, verified safe for this data);
  aggregation uses one-hot matmuls on the PE array. Self loops (PyG
  fill_value='mean') are applied in a per-tile epilogue.
- Layer 1 computes the 17 own tiles (16 grid + poles); xp2 = h1 @ W2ext is
  written to a second table and AllGathered. Layer 2 computes 18 tiles
  (row band plus one halo row on each side; at sample edges the halo is the
  pole-cap row, i.e. the adjacent row rolled by W/2 - the roll lives in the
  host-built index data) so that the 3x3 conv needs no further exchange.
- conv 3x3 (BN folded) + 3-layer MLP run in channel-major (transposed)
  layout; output is [39, 2048] per core, reassembled on the host.
"""
import numpy as np

import concourse.bacc as bacc
import concourse.mybir as mybir
import concourse.tile as tile
from concourse.bass_utils import run_bass_kernel_spmd

# problem constants
B, HH, WW = 2, 64, 128
CX, CCONS = 117, 6
F1 = CX + CCONS
HEADS, HID = 4, 128
HC = HEADS * HID
ED = 8
EMBED = 128
TGT = 39
NCORES = 8
SHARD = 2050
NPAD = NCORES * SHARD
LRELU = 0.01
ATT = 0.2
BN_EPS = 1e-5
STRIDE = 576               # table row f32 elements (2304B, %256 ok)
F32 = mybir.dt.float32
I16 = mybir.dt.int16
AL = mybir.AluOpType

# debug knobs (consulted at build time; bypass _CACHE when using)
DBG_NT = None          # limit number of GAT tiles
DBG_ABLATE = set()     # subset of {"gather","oh","tr","att","act","agg","epi"}


# ---------------------------------------------------------------------------
# host prep
# ---------------------------------------------------------------------------

def _pad_gid(g):
    g = np.asarray(g, np.int64)
    s = g // 8194
    r = g % 8194
    is_pole = r < 2
    i = np.maximum(r - 2, 0)
    row = i // 128
    col = i % 128
    rank = np.where(is_pole, 4 * s, 4 * s + row // 16)
    local = np.where(is_pole, r, 2 + (row % 16) * 128 + col)
    return rank * SHARD + local


def _grid_pad(s, row, col):
    rank = 4 * s + row // 16
    return rank * SHARD + 2 + (row % 16) * 128 + col


def _wrap_idx(flat):
    """[n*128] -> [128, n*8] int16 in dma_gather layout (16-wrapped, x8 replicated)."""
    n = flat.shape[0]
    w = flat.reshape(n // 16, 16).T.astype(np.int16)      # [16, n/16]
    return np.tile(w, (8, 1))


def host_prep(x, x_cons, edge_index, edge_attr, p):
    src_g = edge_index[0].astype(np.int64)
    dst_g = edge_index[1].astype(np.int64)
    src_p = _pad_gid(src_g)
    dst_p = _pad_gid(dst_g)

    d_rank = dst_p // SHARD
    d_local = dst_p % SHARD
    d_is_grid = d_local >= 2
    d_row16 = np.where(d_is_grid, (d_local - 2) // 128, 0)   # row within band
    d_col = np.where(d_is_grid, (d_local - 2) % 128, 0)

    cnt_node = np.zeros(NPAD, np.int64)
    np.add.at(cnt_node, dst_p, 1)

    # ---- layer-1 census: per (rank, tile0..16) ----
    tile1 = np.where(d_is_grid, d_row16, 16)
    cnt1 = np.zeros((NCORES, 17), np.int64)
    np.add.at(cnt1, (d_rank, tile1), 1)
    C1g = int(np.ceil(cnt1[:, :16].max() / 128))
    C1p = max(1, int(np.ceil(cnt1[:, 16].max() / 128)))

    # ---- layer-2 tiles: per core 18 rows (16j-1 .. 16j+16), edge rows rolled ----
    # build per-sample per-row edge lists once
    d_samp = dst_g // 8194
    d_r = dst_g % 8194
    d_isg = d_r >= 2
    d_grow = np.where(d_isg, (d_r - 2) // 128, -1)           # 0..63 or -1 for poles
    d_gcol = np.where(d_isg, (d_r - 2) % 128, 0)
    row_edges = {}
    for s in range(B):
        for r in range(HH):
            row_edges[(s, r)] = np.nonzero((d_samp == s) & (d_grow == r))[0]
    C2 = 1
    core_tiles2 = []
    for k in range(NCORES):
        s, j = k // 4, k % 4
        tiles = []
        for t in range(18):
            row = 16 * j - 1 + t
            rolled = row == -1 or row == HH
            ra = 1 if row == -1 else (HH - 2 if row == HH else row)
            eidx = row_edges[(s, ra)]
            slots = (d_gcol[eidx] - 64) % 128 if rolled else d_gcol[eidx]
            selfrow = [_grid_pad(s, ra, (64 + c) % 128 if rolled else c)
                       for c in range(128)]
            tiles.append((eidx, slots, np.array(selfrow, np.int64)))
            C2 = max(C2, int(np.ceil(len(eidx) / 128)))
        core_tiles2.append(tiles)

    meta = dict(C1g=C1g, C1p=C1p, C2=C2)
    nech1 = 16 * C1g + C1p
    nich1 = 16 * (C1g + 1) + (C1p + 1)
    nech2 = 18 * C2
    nich2 = 18 * (C2 + 1)
    meta.update(nech1=nech1, nich1=nich1, nech2=nech2, nich2=nich2)

    # ---- shared weights ----
    def att_fold(W, att):
        return np.einsum('fhc,hc->fh', W.reshape(-1, HEADS, HID), att).astype(np.float32)

    Wext1 = np.concatenate([p['g1_W'], att_fold(p['g1_W'], p['g1_ad']),
                            att_fold(p['g1_W'], p['g1_as'])], 1).astype(np.float32)
    Wext2 = np.concatenate([p['g2_W'], att_fold(p['g2_W'], p['g2_ad']),
                            att_fold(p['g2_W'], p['g2_as'])], 1).astype(np.float32)
    Me1 = att_fold(p['g1_We'], p['g1_ae'])
    Me2 = att_fold(p['g2_We'], p['g2_ae'])

    s_bn = (p['bn_g'] / np.sqrt(1.0 + BN_EPS)).astype(np.float32)
    convW = (p['conv_w'] * s_bn[:, None, None, None]).astype(np.float32)
    convWT = convW.transpose(2, 3, 1, 0).reshape(3, 3, 4, 128, 128)
    convw_host = np.zeros((128, 36, 128), np.float32)
    for dr in range(3):
        for dc in range(3):
            for ft in range(4):
                convw_host[:, (dr * 3 + dc) * 4 + ft, :] = convWT[dr, dc, ft]
    convb_host = (p['conv_b'] * s_bn + p['bn_b']).astype(np.float32).reshape(128, 1)

    shared = dict(
        wext1=Wext1,                                                    # [123,520]
        wext2=Wext2.reshape(4, 128, 520).transpose(1, 0, 2).copy(),     # [128,4,520]
        me1=Me1, me2=Me2,                                               # [8,4]
        merep1=np.tile(Me1.T[None, :, :], (128, 1, 1)).copy(),          # [128,4,8]
        merep2=np.tile(Me2.T[None, :, :], (128, 1, 1)).copy(),
        brep1=np.tile(p['g1_b'][None, :], (128, 1)).astype(np.float32),
        brep2=np.tile(p['g2_b'][None, :], (128, 1)).astype(np.float32),
        convw=convw_host, convb=convb_host,
        mw1=p['m_w1'].reshape(5, 128, 512).transpose(1, 0, 2).astype(np.float32).copy(),
        mb1t=p['m_b1'].reshape(4, 128).T.astype(np.float32).copy(),
        mw2=p['m_w2'].reshape(4, 128, 256).transpose(1, 0, 2).astype(np.float32).copy(),
        mb2t=p['m_b2'].reshape(2, 128).T.astype(np.float32).copy(),
        mw3=p['m_w3'].reshape(2, 128, TGT).transpose(1, 0, 2).astype(np.float32).copy(),
        mb3c=np.pad(p['m_b3'], (0, 128 - TGT)).astype(np.float32).reshape(128, 1),
        iota_f=np.tile(np.arange(128, dtype=np.float32), (128, 1)),
    )

    feats = np.concatenate([x, x_cons], 1).reshape(B, F1, HH * WW)

    cores = []
    for k in range(NCORES):
        s, j = k // 4, k % 4
        # ---- layer 1 arrays ----
        g1 = np.zeros(nich1 * 128, np.int64)
        d1 = np.zeros(nech1 * 128, np.int64)
        dl1 = np.full(nech1 * 128, -1.0, np.float32)
        ea1 = np.zeros((nech1 * 128, ED), np.float32)
        ic = ec = 0
        for t in range(17):
            C = C1g if t < 16 else C1p
            if t < 16:
                m = (d_rank == k) & d_is_grid & (d_row16 == t)
            else:
                m = (d_rank == k) & ~d_is_grid
            eidx = np.nonzero(m)[0]
            n = len(eidx)
            assert n <= C * 128
            g1[ic + np.arange(n)] = src_p[eidx]
            d1[ec + np.arange(n)] = dst_p[eidx]
            dl1[ec + np.arange(n)] = d_col[eidx] if t < 16 else d_local[eidx]
            ea1[ec + np.arange(n)] = edge_attr[eidx]
            base = k * SHARD + (2 + 128 * t if t < 16 else 0)
            g1[ic + C * 128: ic + (C + 1) * 128] = base + np.arange(128)
            ic += (C + 1) * 128
            ec += C * 128
        cnti1 = np.ones((128, 17), np.float32)
        for t in range(16):
            b0 = k * SHARD + 2 + 128 * t
            cnti1[:, t] = 1.0 / np.maximum(cnt_node[b0:b0 + 128], 1)
        cnti1[:, 16] = 1.0 / np.maximum(cnt_node[k * SHARD:k * SHARD + 128], 1)

        # ---- layer 2 arrays ----
        g2 = np.zeros(nich2 * 128, np.int64)
        d2 = np.zeros(nech2 * 128, np.int64)
        dl2 = np.full(nech2 * 128, -1.0, np.float32)
        ea2 = np.zeros((nech2 * 128, ED), np.float32)
        cnti2 = np.ones((128, 18), np.float32)
        ic = ec = 0
        for t in range(18):
            eidx, slots, selfrow = core_tiles2[k][t]
            n = len(eidx)
            g2[ic + np.arange(n)] = src_p[eidx]
            d2[ec + np.arange(n)] = dst_p[eidx]
            dl2[ec + np.arange(n)] = slots
            ea2[ec + np.arange(n)] = edge_attr[eidx]
            g2[ic + C2 * 128: ic + (C2 + 1) * 128] = selfrow
            cnti2[:, t] = 1.0 / np.maximum(cnt_node[selfrow], 1)
            ic += (C2 + 1) * 128
            ec += C2 * 128

        xT = np.zeros((F1, SHARD), np.float32)
        xT[:, 2:] = feats[s, :, j * 2048:(j + 1) * 2048]
        resid = x[s, -TGT:].reshape(TGT, HH * WW)[:, j * 2048:(j + 1) * 2048]

        core = dict(
            xT=xT,
            gidx1=_wrap_idx(g1), gidx2=_wrap_idx(g2),
            didx1=_wrap_idx(d1), didx2=_wrap_idx(d2),
            dloc1=dl1.reshape(nech1, 128).T.copy(),
            dloc2=dl2.reshape(nech2, 128).T.copy(),
            ea1=ea1.reshape(nech1, 128, ED).transpose(1, 0, 2).reshape(128, nech1 * ED).copy(),
            ea2=ea2.reshape(nech2, 128, ED).transpose(1, 0, 2).reshape(128, nech2 * ED).copy(),
            eaT1=ea1.T.copy(), eaT2=ea2.T.copy(),
            cnti1=cnti1, cnti2=cnti2,
            resid=np.ascontiguousarray(resid.astype(np.float32)),
        )
        core.update(shared)
        cores.append(core)
    return cores, meta


# ---------------------------------------------------------------------------
# device program
# ---------------------------------------------------------------------------

def _leaky(nc, pool, out_ap, in_ap, alpha):
    """out = leaky(in) via mult (ACT) + max (DVE); Lrelu not sim-supported."""
    shp = [in_ap.shape[0], int(np.prod(in_ap.shape[1:]))]
    t2 = pool.tile(shp, F32, tag="lk_tmp")
    nc.scalar.mul(t2[:], in_ap, alpha)
    nc.vector.tensor_tensor(out_ap, in_ap, t2[:], op=AL.max)


def build_program(meta, stage="full"):
    C1g, C1p, C2 = meta['C1g'], meta['C1p'], meta['C2']
    nech1, nich1 = meta['nech1'], meta['nich1']
    nech2, nich2 = meta['nech2'], meta['nich2']

    nc = bacc.Bacc("TRN2", target_bir_lowering=False, debug=False,
                   num_devices=NCORES)
    dbg = {}

    def din(name, shape, dt=F32):
        return nc.dram_tensor(name, shape, dt, kind="ExternalInput")

    xT_d = din("xT", [F1, SHARD])
    gidx1_d = din("gidx1", [128, nich1 * 8], I16)
    gidx2_d = din("gidx2", [128, nich2 * 8], I16)
    didx1_d = din("didx1", [128, nech1 * 8], I16)
    didx2_d = din("didx2", [128, nech2 * 8], I16)
    dloc1_d = din("dloc1", [128, nech1])
    dloc2_d = din("dloc2", [128, nech2])
    ea1_d = din("ea1", [128, nech1 * ED])
    ea2_d = din("ea2", [128, nech2 * ED])
    eaT1_d = din("eaT1", [ED, nech1 * 128])
    eaT2_d = din("eaT2", [ED, nech2 * 128])
    cnti1_d = din("cnti1", [128, 17])
    cnti2_d = din("cnti2", [128, 18])
    resid_d = din("resid", [TGT, 2048])
    wext1_d = din("wext1", [F1, 520])
    wext2_d = din("wext2", [128, 4, 520])
    me1_d = din("me1", [ED, 4])
    me2_d = din("me2", [ED, 4])
    merep1_d = din("merep1", [128, 4, ED])
    merep2_d = din("merep2", [128, 4, ED])
    brep1_d = din("brep1", [128, HC])
    brep2_d = din("brep2", [128, HC])
    convw_d = din("convw", [128, 36, 128])
    convb_d = din("convb", [128, 1])
    mw1_d = din("mw1", [128, 5, 512])
    mb1t_d = din("mb1t", [128, 4])
    mw2_d = din("mw2", [128, 4, 256])
    mb2t_d = din("mb2t", [128, 2])
    mw3_d = din("mw3", [128, 2, TGT])
    mb3c_d = din("mb3c", [128, 1])
    iota_d = din("iota_f", [128, 128])
    out_d = nc.dram_tensor("out", [TGT, 2048], F32, kind="ExternalOutput")
    if stage in ("p0",):
        dbg['tab1'] = nc.dram_tensor("dbg_tab1", [NPAD, STRIDE], F32, kind="ExternalOutput")
    if stage in ("g1", "g1x", "g1t"):
        dbg['h1'] = nc.dram_tensor("dbg_h1", [128, 17, HC], F32, kind="ExternalOutput")
    tab1_in_d = None
    if stage in ("g1x", "g1t"):
        tab1_in_d = nc.dram_tensor("tab1_in", [NPAD, STRIDE], F32, kind="ExternalInput")
    if stage in ("pb",):
        dbg['tab2'] = nc.dram_tensor("dbg_tab2", [NPAD, STRIDE], F32, kind="ExternalOutput")
    if stage in ("g2",):
        dbg['prows'] = nc.dram_tensor("dbg_prows", [128, 4, 18, 130], F32, kind="ExternalOutput")

    with tile.TileContext(nc) as tc:
        with (
            tc.tile_pool(name="dram", bufs=1, space="DRAM") as dram,
            tc.tile_pool(name="consts", bufs=1) as consts,
            tc.tile_pool(name="persist", bufs=1) as persist,
        ):
            tab1_shard = dram.tile([SHARD, STRIDE], F32)
            tab1_full = dram.tile([NPAD, STRIDE], F32, addr_space="Shared")
            tab2_shard = dram.tile([SHARD, STRIDE], F32)
            tab2_full = dram.tile([NPAD, STRIDE], F32, addr_space="Shared")

            iota_t = consts.tile([128, 128], F32)
            nc.sync.dma_start(iota_t[:], iota_d[:])
            iota_pi = consts.tile([128, 1], mybir.dt.int32)
            nc.gpsimd.iota(iota_pi[:], [[0, 1]], base=0, channel_multiplier=1)
            iota_p = consts.tile([128, 1], F32)
            nc.vector.tensor_copy(iota_p[:], iota_pi[:])
            ident = consts.tile([128, 128], F32)
            nc.vector.tensor_scalar(ident[:], iota_t[:], iota_p[:], None,
                                    op0=AL.is_equal)

            # prows: padded conv input rows, [p, ftile, 18 rows, 130 cols]
            prows = persist.tile([128, 4, 18, 130], F32)

            # ---------------- phase 0: table 1 ----------------
            if stage not in ("g1x", "g1t"):
             with (
                 tc.tile_pool(name="p0", bufs=2) as p0,
                 tc.tile_pool(name="p0ps", bufs=2, space="PSUM") as p0ps,
             ):
                 xT_sb = p0.tile([F1, SHARD], F32, tag="xT")
                 nc.sync.dma_start(xT_sb[:], xT_d[:])
                 w1_sb = p0.tile([F1, 520], F32, tag="w1")
                 nc.sync.dma_start(w1_sb[:], wext1_d[:])
                 for t in range(17):
                     c0 = 2 + 128 * t if t < 16 else 0
                     ps = p0ps.tile([128, 520], F32, tag="ps0")
                     nc.tensor.matmul(ps[:, 0:512], xT_sb[:, c0:c0 + 128],
                                      w1_sb[:, 0:512], start=True, stop=True)
                     nc.tensor.matmul(ps[:, 512:520], xT_sb[:, c0:c0 + 128],
                                      w1_sb[:, 512:520], start=True, stop=True)
                     w = p0.tile([128, STRIDE], F32, tag="wt")
                     nc.vector.memset(w[:, 520:STRIDE], 0.0)
                     nc.vector.tensor_copy(w[:, 0:520], ps[:])
                     if t < 16:
                         nc.sync.dma_start(tab1_shard[c0:c0 + 128, :], w[:])
                     else:
                         nc.sync.dma_start(tab1_shard[0:2, :], w[0:2, :])

            if stage not in ("g1x", "g1t"):
                nc.gpsimd.collective_compute(
                    "AllGather", AL.bypass, replica_groups=[list(range(NCORES))],
                    ins=[tab1_shard.opt()], outs=[tab1_full.opt()])
            elif stage == "g1x":
                nc.gpsimd.dma_start(tab1_full[:], tab1_in_d[:])
            tab1_src = tab1_in_d if stage == "g1t" else tab1_full

            if stage == "p0":
                nc.gpsimd.dma_start(dbg['tab1'][:], tab1_full[:])
            bail = stage == "p0"

            # ---------------- GAT layer (generic) ----------------
            def gat_layer(tab_src, ntiles, Cs, gidx_sb, didx_sb, dloc_sb,
                          ea_sb, eaT_d, cnti_sb, me_sb, merep_sb, brep_sb,
                          gp, psA, psB, out_writer):
                ic = ec = 0
                for t in range(ntiles if DBG_NT is None else min(DBG_NT, ntiles)):
                    C = Cs[t]
                    g = gp.tile([128, C + 1, STRIDE], F32, tag="grow", bufs=3)
                    nc.gpsimd.dma_gather(
                        g[:], tab_src[:], gidx_sb[:, ic * 8:(ic + C + 1) * 8],
                        (C + 1) * 128, (C + 1) * 128, STRIDE,
                        single_packet=False)
                    gd = gp.tile([128, C, 64], F32, tag="gdst")
                    nc.gpsimd.dma_gather(
                        gd[:], tab_src[:, 512:STRIDE], didx_sb[:, ec * 8:(ec + C) * 8],
                        C * 128, C * 128, 64, elem_step=STRIDE,
                        single_packet=False)
                    eaTt = gp.tile([ED, C * 128], F32, tag="eaT")
                    nc.sync.dma_start(eaTt[:], eaT_d[:, ec * 128:(ec + C) * 128])
                    oh_w = gp.tile([128, C, 128], F32, tag="ohw")
                    ps_att = psB.tile([128, 4 * C], F32, tag="patt")
                    ps_out = psA.tile([128, HC], F32, tag="pout")
                    ps_aux = psB.tile([128, 16], F32, tag="paux")
                    # pass 1: one-hots (GPSIMD) + per-edge a_edge into wide psum
                    for c in range(C):
                        nc.gpsimd.tensor_scalar(oh_w[:, c, :], iota_t[:],
                                                dloc_sb[:, ec + c:ec + c + 1],
                                                None, op0=AL.is_equal)
                        nc.tensor.matmul(ps_att[:, c * 4:(c + 1) * 4],
                                         eaTt[:, c * 128:(c + 1) * 128], me_sb[:],
                                         start=(c == 0), stop=(c == C - 1))
                    # wide attention: alpha = a_edge + a_src[src] + a_dst[dst]
                    alp = gp.tile([128, C, 4], F32, tag="alpw")
                    nc.vector.tensor_tensor(
                        alp[:], ps_att[:].rearrange("p (c f) -> p c f", f=4),
                        g[:, 0:C, 516:520], op=AL.add)
                    nc.vector.tensor_tensor(alp[:], alp[:], gd[:, :, 0:4], op=AL.add)
                    alw = gp.tile([128, C, 4], F32, tag="alw")
                    nc.scalar.mul(alw[:], alp[:], ATT)
                    nc.vector.tensor_tensor(alw[:], alw[:], alp[:], op=AL.max)
                    exw = gp.tile([128, C, 4], F32, tag="exw")
                    nc.scalar.activation(exw[:], alw[:],
                                         mybir.ActivationFunctionType.Exp)
                    # pass 2: weighted messages, aggregated via one-hot matmuls
                    for c in range(C):
                        gm = gp.tile([128, HC], F32, tag="gm")
                        nc.scalar.activation(gm[:, 0:HID], g[:, c, 0:HID],
                                             mybir.ActivationFunctionType.Copy,
                                             scale=exw[:, c, 0:1])
                        for h in (1, 2, 3):
                            nc.vector.tensor_scalar(
                                gm[:, h * HID:(h + 1) * HID],
                                g[:, c, h * HID:(h + 1) * HID],
                                exw[:, c, h:h + 1], None, op0=AL.mult)
                        last = c == C - 1
                        nc.tensor.matmul(ps_out[:], oh_w[:, c, :], gm[:],
                                         start=(c == 0), stop=last)
                        nc.tensor.matmul(ps_aux[:, 0:4], oh_w[:, c, :],
                                         exw[:, c, :], start=(c == 0), stop=False)
                        nc.tensor.matmul(ps_aux[:, 4:12], oh_w[:, c, :],
                                         ea_sb[:, (ec + c) * ED:(ec + c + 1) * ED],
                                         start=False, stop=last)
                    # ---- epilogue (self loops, softmax normalize) ----
                    la = gp.tile([128, ED], F32, tag="la")
                    nc.vector.tensor_scalar(la[:], ps_aux[:, 4:12],
                                            cnti_sb[:, t:t + 1], None, op0=AL.mult)
                    ael = gp.tile([128, 4], F32, tag="ael")
                    scr = gp.tile([128, ED], F32, tag="scr")
                    for h in range(HEADS):
                        nc.vector.tensor_tensor(scr[:], la[:],
                                                merep_sb[:, h, :], op=AL.mult)
                        nc.vector.reduce_sum(ael[:, h:h + 1], scr[:],
                                             axis=mybir.AxisListType.X)
                    alp2 = gp.tile([128, 4], F32, tag="alp2")
                    nc.vector.tensor_tensor(alp2[:], g[:, C, 516:520],
                                            g[:, C, 512:516], op=AL.add)
                    nc.vector.tensor_tensor(alp2[:], alp2[:], ael[:], op=AL.add)
                    all_ = gp.tile([128, 4], F32, tag="all")
                    nc.scalar.mul(all_[:], alp2[:], ATT)
                    nc.vector.tensor_tensor(all_[:], all_[:], alp2[:], op=AL.max)
                    exl = gp.tile([128, 4], F32, tag="exl")
                    nc.scalar.activation(exl[:], all_[:],
                                         mybir.ActivationFunctionType.Exp)
                    den = gp.tile([128, 4], F32, tag="den")
                    nc.vector.tensor_tensor(den[:], ps_aux[:, 0:4], exl[:], op=AL.add)
                    rec = gp.tile([128, 4], F32, tag="rec")
                    nc.vector.reciprocal(rec[:], den[:])
                    exr = gp.tile([128, 4], F32, tag="exr")
                    nc.vector.tensor_tensor(exr[:], exl[:], rec[:], op=AL.mult)
                    hsA = gp.tile([128, HC], F32, tag="hsA")
                    hsB = gp.tile([128, HC], F32, tag="hsB")
                    for h in range(HEADS):
                        nc.scalar.activation(
                            hsA[:, h * HID:(h + 1) * HID],
                            ps_out[:, h * HID:(h + 1) * HID],
                            mybir.ActivationFunctionType.Copy,
                            scale=rec[:, h:h + 1])
                        nc.vector.tensor_scalar(
                            hsB[:, h * HID:(h + 1) * HID],
                            g[:, C, h * HID:(h + 1) * HID],
                            exr[:, h:h + 1], None, op0=AL.mult)
                    hb0 = gp.tile([128, HC], F32, tag="hb0")
                    nc.gpsimd.tensor_tensor(hb0[:], hsA[:], hsB[:], op=AL.add)
                    hb = gp.tile([128, HC], F32, tag="hb")
                    nc.vector.tensor_tensor(hb[:], hb0[:], brep_sb[:], op=AL.add)
                    out_writer(t, hb, gp)
                    ic += C + 1
                    ec += C

            # ---------------- GAT 1 + table 2 (h1 lives across both) ----------
            if not bail:
             with tc.tile_pool(name="ph1", bufs=1) as ph1:
              h1_all = ph1.tile([128, 17, HC], F32)
              with (
                tc.tile_pool(name="g1", bufs=2) as gp1,
                tc.tile_pool(name="g1c", bufs=1) as gc1,
                tc.tile_pool(name="g1psA", bufs=2, space="PSUM") as ps1A,
                tc.tile_pool(name="g1psB", bufs=2, space="PSUM") as ps1B,
              ):
                gidx1_sb = gc1.tile([128, nich1 * 8], I16)
                nc.sync.dma_start(gidx1_sb[:], gidx1_d[:])
                didx1_sb = gc1.tile([128, nech1 * 8], I16)
                nc.sync.dma_start(didx1_sb[:], didx1_d[:])
                dloc1_sb = gc1.tile([128, nech1], F32)
                nc.sync.dma_start(dloc1_sb[:], dloc1_d[:])
                ea1_sb = gc1.tile([128, nech1 * ED], F32)
                nc.sync.dma_start(ea1_sb[:], ea1_d[:])
                cnti1_sb = gc1.tile([128, 17], F32)
                nc.sync.dma_start(cnti1_sb[:], cnti1_d[:])
                me1_sb = gc1.tile([ED, 4], F32)
                nc.sync.dma_start(me1_sb[:], me1_d[:])
                merep1_sb = gc1.tile([128, 4, ED], F32)
                nc.sync.dma_start(merep1_sb[:], merep1_d[:])
                brep1_sb = gc1.tile([128, HC], F32)
                nc.sync.dma_start(brep1_sb[:], brep1_d[:])

                def h1_writer(t, hb, gp):
                    _leaky(nc, gp, h1_all[:, t, :], hb[:], LRELU)

                gat_layer(tab1_src, 17, [C1g] * 16 + [C1p], gidx1_sb, didx1_sb,
                          dloc1_sb, ea1_sb, eaT1_d, cnti1_sb, me1_sb, merep1_sb,
                          brep1_sb, gp1, ps1A, ps1B, h1_writer)
                if stage in ("g1", "g1x", "g1t"):
                    nc.sync.dma_start(dbg['h1'][:], h1_all[:])

              # ---------------- phase B: table 2 ----------------
              if stage not in ("g1", "g1x", "g1t"):
               with (
                 tc.tile_pool(name="pb", bufs=2) as pb,
                 tc.tile_pool(name="pbc", bufs=1) as pbc,
                 tc.tile_pool(name="pbps", bufs=2, space="PSUM") as pbps,
               ):
                 w2_sb = pbc.tile([128, 4, 520], F32)
                 nc.sync.dma_start(w2_sb[:], wext2_d[:])
                 for t in range(17):
                     h1T = pb.tile([128, 4, 128], F32, tag="h1T")
                     for kt in range(4):
                         ptr = pbps.tile([128, 128], F32, tag="ptrB")
                         nc.tensor.transpose(ptr[:], h1_all[:, t, kt * 128:(kt + 1) * 128],
                                             ident[:])
                         nc.vector.tensor_copy(h1T[:, kt, :], ptr[:])
                     ps = pbps.tile([128, 520], F32, tag="psB")
                     for kt in range(4):
                         nc.tensor.matmul(ps[:, 0:512], h1T[:, kt, :],
                                          w2_sb[:, kt, 0:512],
                                          start=(kt == 0), stop=(kt == 3))
                         nc.tensor.matmul(ps[:, 512:520], h1T[:, kt, :],
                                          w2_sb[:, kt, 512:520],
                                          start=(kt == 0), stop=(kt == 3))
                     w = pb.tile([128, STRIDE], F32, tag="wtB")
                     nc.vector.memset(w[:, 520:STRIDE], 0.0)
                     nc.vector.tensor_copy(w[:, 0:520], ps[:])
                     if t < 16:
                         nc.sync.dma_start(tab2_shard[2 + 128 * t:2 + 128 * t + 128, :], w[:])
                     else:
                         nc.sync.dma_start(tab2_shard[0:2, :], w[0:2, :])

            bail = bail or stage in ("g1", "g1x", "g1t")
            if not bail:
                nc.gpsimd.collective_compute(
                    "AllGather", AL.bypass, replica_groups=[list(range(NCORES))],
                    ins=[tab2_shard.opt()], outs=[tab2_full.opt()])
                if stage == "pb":
                    nc.gpsimd.dma_start(dbg['tab2'][:], tab2_full[:])
            bail = bail or stage == "pb"

            # ---------------- GAT 2 (18 tiles incl. halo) ----------------
            if not bail:
             with (
                 tc.tile_pool(name="g2", bufs=2) as gp2,
                 tc.tile_pool(name="g2c", bufs=1) as gc2,
                 tc.tile_pool(name="g2psA", bufs=2, space="PSUM") as ps2A,
                 tc.tile_pool(name="g2psB", bufs=2, space="PSUM") as ps2B,
             ):
                 gidx2_sb = gc2.tile([128, nich2 * 8], I16)
                 nc.sync.dma_start(gidx2_sb[:], gidx2_d[:])
                 didx2_sb = gc2.tile([128, nech2 * 8], I16)
                 nc.sync.dma_start(didx2_sb[:], didx2_d[:])
                 dloc2_sb = gc2.tile([128, nech2], F32)
                 nc.sync.dma_start(dloc2_sb[:], dloc2_d[:])
                 ea2_sb = gc2.tile([128, nech2 * ED], F32)
                 nc.sync.dma_start(ea2_sb[:], ea2_d[:])
                 cnti2_sb = gc2.tile([128, 18], F32)
                 nc.sync.dma_start(cnti2_sb[:], cnti2_d[:])
                 me2_sb = gc2.tile([ED, 4], F32)
                 nc.sync.dma_start(me2_sb[:], me2_d[:])
                 merep2_sb = gc2.tile([128, 4, ED], F32)
                 nc.sync.dma_start(merep2_sb[:], merep2_d[:])
                 brep2_sb = gc2.tile([128, HC], F32)
                 nc.sync.dma_start(brep2_sb[:], brep2_d[:])

                 def h2_writer(t, hb, gp):
                     h2t = gp.tile([128, HC], F32, tag="h2t")
                     _leaky(nc, gp, h2t[:], hb[:], LRELU)
                     for kt in range(4):
                         ptr = ps2A.tile([128, 128], F32, tag="ptr")
                         nc.tensor.transpose(ptr[:], h2t[:, kt * 128:(kt + 1) * 128],
                                             ident[:])
                         nc.vector.tensor_copy(prows[:, kt, t, 1:129], ptr[:])
                     nc.vector.tensor_copy(prows[:, :, t, 0:1], prows[:, :, t, 128:129])
                     nc.vector.tensor_copy(prows[:, :, t, 129:130], prows[:, :, t, 1:2])

                 gat_layer(tab2_full, 18, [C2] * 18, gidx2_sb, didx2_sb,
                           dloc2_sb, ea2_sb, eaT2_d, cnti2_sb, me2_sb, merep2_sb,
                           brep2_sb, gp2, ps2A, ps2B, h2_writer)

            if stage == "g2":
                nc.sync.dma_start(dbg['prows'][:], prows[:])
            bail = bail or stage == "g2"

            # ---------------- conv 3x3 ----------------
            if not bail:
             nl = persist.tile([128, 4, 512], F32)
             with (
                 tc.tile_pool(name="cv", bufs=2) as cv,
                 tc.tile_pool(name="cvc", bufs=1) as cvc,
                 tc.tile_pool(name="cvps", bufs=1, space="PSUM") as cvps,
             ):
                 convw_sb = cvc.tile([128, 36, 128], F32)
                 nc.sync.dma_start(convw_sb[:], convw_d[:])
                 convb_sb = cvc.tile([128, 1], F32)
                 nc.sync.dma_start(convb_sb[:], convb_d[:])
                 pg = []
                 for gi in range(4):
                     pgt = cvps.tile([128, 512], F32, tag=f"cv{gi}", name=f"pgcv{gi}")
                     pg.append(pgt)
                 for ci in range(36):
                     dr, dc, ft = ci // 12, (ci // 4) % 3, ci % 4
                     for gi in range(4):
                         nc.tensor.matmul(
                             pg[gi][:], convw_sb[:, ci, :],
                             prows[:, ft, 4 * gi + dr:4 * gi + dr + 4, dc:dc + 128],
                             start=(ci == 0), stop=(ci == 35))
                 for gi in range(4):
                     nlb = cv.tile([128, 512], F32, tag="nlb")
                     nc.scalar.activation(nlb[:], pg[gi][:],
                                          mybir.ActivationFunctionType.Identity,
                                          bias=convb_sb[:, 0:1])
                     _leaky(nc, cv, nl[:, gi, :], nlb[:], LRELU)

             # ---------------- MLP ----------------
             with (
                 tc.tile_pool(name="ml", bufs=2) as ml,
                 tc.tile_pool(name="mlc", bufs=1) as mlc,
                 tc.tile_pool(name="mlp2", bufs=1) as mlpers,
                 tc.tile_pool(name="mlps", bufs=2, space="PSUM") as mlps,
             ):
                 mw1_sb = mlc.tile([128, 5, 512], F32)
                 nc.sync.dma_start(mw1_sb[:], mw1_d[:])
                 mb1t_sb = mlc.tile([128, 4], F32)
                 nc.sync.dma_start(mb1t_sb[:], mb1t_d[:])
                 mw2_sb = mlc.tile([128, 4, 256], F32)
                 nc.sync.dma_start(mw2_sb[:], mw2_d[:])
                 mb2t_sb = mlc.tile([128, 2], F32)
                 nc.sync.dma_start(mb2t_sb[:], mb2t_d[:])
                 mw3_sb = mlc.tile([128, 2, TGT], F32)
                 nc.sync.dma_start(mw3_sb[:], mw3_d[:])
                 mb3c_sb = mlc.tile([128, 1], F32)
                 nc.sync.dma_start(mb3c_sb[:], mb3c_d[:])
                 resid_sb = mlc.tile([TGT, 2048], F32)
                 nc.sync.dma_start(resid_sb[:], resid_d[:])

                 o1 = mlpers.tile([128, 4, 2048], F32)
                 o2 = mlpers.tile([128, 2, 2048], F32)
                 outsb = mlpers.tile([TGT, 2048], F32)

                 for og in range(4):
                     for pc in range(4):
                         pm = mlps.tile([128, 512], F32, tag="pm1")
                         for kt in range(4):
                             nc.tensor.matmul(
                                 pm[:], mw1_sb[:, kt, og * 128:(og + 1) * 128],
                                 prows[:, kt, 1 + 4 * pc:1 + 4 * pc + 4, 1:129],
                                 start=(kt == 0), stop=False)
                         nc.tensor.matmul(pm[:], mw1_sb[:, 4, og * 128:(og + 1) * 128],
                                          nl[:, pc, :], start=False, stop=True)
                         ob = ml.tile([128, 512], F32, tag="ob1")
                         nc.scalar.activation(ob[:], pm[:],
                                              mybir.ActivationFunctionType.Identity,
                                              bias=mb1t_sb[:, og:og + 1])
                         _leaky(nc, ml, o1[:, og, pc * 512:(pc + 1) * 512], ob[:], LRELU)
                 for og in range(2):
                     for pc in range(4):
                         pm = mlps.tile([128, 512], F32, tag="pm2")
                         for kt in range(4):
                             nc.tensor.matmul(
                                 pm[:], mw2_sb[:, kt, og * 128:(og + 1) * 128],
                                 o1[:, kt, pc * 512:(pc + 1) * 512],
                                 start=(kt == 0), stop=(kt == 3))
                         ob = ml.tile([128, 512], F32, tag="ob2")
                         nc.scalar.activation(ob[:], pm[:],
                                              mybir.ActivationFunctionType.Identity,
                                              bias=mb2t_sb[:, og:og + 1])
                         _leaky(nc, ml, o2[:, og, pc * 512:(pc + 1) * 512], ob[:], LRELU)
                 for pc in range(4):
                     pm = mlps.tile([TGT, 512], F32, tag="pm3")
                     for kt in range(2):
                         nc.tensor.matmul(pm[:], mw3_sb[:, kt, :],
                                          o2[:, kt, pc * 512:(pc + 1) * 512],
                                          start=(kt == 0), stop=(kt == 1))
                     ob = ml.tile([TGT, 512], F32, tag="ob3")
                     nc.vector.tensor_scalar(ob[:], pm[:], mb3c_sb[0:TGT, :], None,
                                             op0=AL.add)
                     nc.vector.tensor_tensor(outsb[:, pc * 512:(pc + 1) * 512], ob[:],
                                             resid_sb[:, pc * 512:(pc + 1) * 512],
                                             op=AL.add)
                 nc.sync.dma_start(out_d[:], outsb[:])

    nc.compile()
    return nc


# ---------------------------------------------------------------------------
# runner
# ---------------------------------------------------------------------------

_CACHE = {}


def _get_program(meta, stage="full"):
    key = (stage,) + tuple(sorted(meta.items()))
    if key not in _CACHE:
        _CACHE[key] = build_program(meta, stage)
    return _CACHE[key]


def run(inputs, trace=False, stage="full", **kw):
    x = np.asarray(inputs['x'], np.float32)
    x_cons = np.asarray(inputs['x_cons'], np.float32)
    edge_index = np.asarray(inputs['edge_index'])
    edge_attr = np.asarray(inputs['edge_attr'], np.float32)
    params = {k: np.asarray(v, np.float32) for k, v in inputs.items()
              if k not in ('x', 'x_cons', 'edge_index', 'edge_attr')}
    cores, meta = host_prep(x, x_cons, edge_index, edge_attr, params)
    nc = _get_program(meta, stage)
    res = run_bass_kernel_spmd(nc, cores, core_ids=list(range(NCORES)),
                               trace=trace, **kw)
    out = np.zeros((B, TGT, HH, WW), np.float32)
    for k in range(NCORES):
        s, j = k // 4, k % 4
        out[s, :, 16 * j:16 * j + 16, :] = \
            res.results[k]["out"].reshape(TGT, 16, WW)
    return out, res


def kernel(**inputs) -> np.ndarray:
    out, _ = run(inputs)
    return out

